# revision 1
# baseline (speedup 1.0000x reference)
"""Trainium2 Bass kernel for the Hebbian fast-weight memory module.

Reference computation (B=256 batches, T=16 steps, M=256):
    step t:  p2 = learn * relu6(learn2*x_t + A @ x_t)
             A  = (1-decay)*A + outer(x_t, p2)
    output:  relu6(A_final @ x_query)

Key identity:
    A_t = g^{t+1} A_init + sum_{s<=t} g^{t-s} * outer(x_s, p2_s),   g = 1-decay
so with Phi_s = relu6(learn2*x_s + y_s)  (p2_s = learn*Phi_s):
    y_t[i] = (A_{t-1} @ x_t)[i]
           = g^t (A_init@x_t)[i] + sum_{s<t} g^{t-1-s} learn (Phi_s . x_t) x_s[i]
    out[i] = relu6( g^16 (A_init@x_q)[i] + sum_s g^{15-s} learn (Phi_s . x_q) x_s[i] )
A is never materialized on device. The A_init matvec terms are host-precomputed
(numpy) and folded into the additive tensors -- they are exactly zero for the
spec's A_init==0, so the host does no einsum work in the graded path.

Sharding: batch 256 -> 8 cores x 32 batches (pure data parallel, no collectives).

On-chip layout per core (bpc=32 batches):
  partitions p = s4*32 + b  (s4 in [0,4), b in [0,32)), history step s = 4k+s4
  PH[k]  [128,256]  Phi history tile k (k=0..3), zero-init, row-block per step
  XB[t]  [128,256]  x_t replicated over s4 (t=16 -> x_query)     (host-prepped)
  XHW[k] [128,256]  learn * g^{-(s+1)} * x_s                     (host-prepped)
  SELW[t][128, 32]  g^t * one-hot(b)                             (host-prepped)
  ADD[t] [ 32,256]  learn2*x_t + g^t*(A_init@x_t); t=16 variant  (host-prepped)

Per step t: 4x tensor_tensor_reduce (DVE): cw_k = sum_m PH[k]*XB[t]
            4x activation-Copy (ACT):      selcw_k = SELW[t] * cw_k (bcast)
            4x matmul (PE, accumulate):    y += selcw_k.T @ XHW[k]   -> [32,256]
            z = y + ADD[t] (DVE); PH[t//4][rows t%4] = clip(z,0,6) (DVE)
Final: same stage against x_query, then out = clip(y+ADD[16],0,6) -> DMA out.
All input-data-dependent values arrive via DRAM tensors (host numpy), so no
input data is baked into the compiled NEFF.
"""

import os
import sys

for _p in ("/opt/pypackages", "/opt/trn_rl_repo"):
    if _p not in sys.path:
        sys.path.insert(0, _p)

import numpy as np

B, T, M = 256, 16, 256
NCORES = 8
BPC = B // NCORES  # 32 batches per core
NSTILE = 4         # history tiles; each holds 4 steps x 32 batches = 128 partitions

_COMPILED = {}


def _build_program(dots_dtype, fused_relu6=True):
    import concourse.bacc as bacc
    import concourse.mybir as mybir
    from concourse.tile import TileContext

    f32 = mybir.dt.float32
    bf16 = mybir.dt.bfloat16
    Alu = mybir.AluOpType
    Act = mybir.ActivationFunctionType

    nc = bacc.Bacc(target_bir_lowering=False)

    xb_d = nc.dram_tensor("xb", [128, (T + 1) * M], dots_dtype,
                          kind="ExternalInput")
    eye_d = nc.dram_tensor("eye", [BPC, BPC], bf16, kind="ExternalInput")
    xhw_d = nc.dram_tensor("xhw", [128, NSTILE * M], bf16, kind="ExternalInput")
    selw_d = nc.dram_tensor("selw", [128, (T + 1) * BPC], bf16, kind="ExternalInput")
    add_d = nc.dram_tensor("addt", [BPC, (T + 1) * M],
                           bf16 if fused_relu6 else f32, kind="ExternalInput")
    out_d = nc.dram_tensor("out", [BPC, M], f32, kind="ExternalOutput")

    with TileContext(nc) as tc:
        with (
            tc.tile_pool(name="persist", bufs=1) as pp,
            tc.tile_pool(name="work", bufs=8) as wp,
            tc.tile_pool(name="psum", bufs=6, space="PSUM") as psp,
        ):
            xb_all = pp.tile([128, (T + 1) * M], dots_dtype, tag="xb",
                             name="xb_sb")
            xb = [xb_all[:, t * M:(t + 1) * M] for t in range(T + 1)]
            eye_sb = pp.tile([BPC, BPC], bf16, tag="eye", name="eye_sb")
            xhw_all = pp.tile([128, NSTILE * M], bf16, tag="xhw", name="xhw_sb")
            xhw = [xhw_all[:, k * M:(k + 1) * M] for k in range(NSTILE)]
            selw_all = pp.tile([128, (T + 1) * BPC], bf16, tag="selw",
                               name="selw_sb")
            selw = [selw_all[:, t * BPC:(t + 1) * BPC] for t in range(T + 1)]
            addt_all = pp.tile([BPC, (T + 1) * M],
                               bf16 if fused_relu6 else f32, tag="addt",
                               name="addt_sb")
            addt = [addt_all[:, t * M:(t + 1) * M] for t in range(T + 1)]
            ph = [pp.tile([128, M], dots_dtype, tag=f"ph{k}", name=f"ph{k}")
                  for k in range(NSTILE)]

            XB_SPLIT = 5 * M
            nc.scalar.dma_start(out=xb_all[:, :XB_SPLIT],
                                in_=xb_d[:, :XB_SPLIT])
            nc.scalar.dma_start(out=addt_all[:], in_=add_d[:, :])
            nc.sync.dma_start(out=selw_all[:], in_=selw_d[:, :])
            nc.sync.dma_start(out=xhw_all[:], in_=xhw_d[:, :])
            nc.sync.dma_start(out=eye_sb[:], in_=eye_d[:, :])
            nc.sync.dma_start(out=xb_all[:, XB_SPLIT:], in_=xb_d[:, XB_SPLIT:])

            for k in range(NSTILE):
                nc.vector.memset(ph[k][:], 0.0)

            def step_y(t):
                """cw_k = rowdot(relu6(PH[k]), XB[t]); y += (SELW*cw_k).T @ XHW[k].

                PH stores PRE-activation z; relu6 folds into the dot as
                op0=min(.,6) (valid when z >= 0: A_init==0 and inputs >= 0,
                checked host-side; fused_relu6=False builds the general
                variant). Only history tiles with data (k <= (t-1)//4) are
                processed; the hottest k goes last so the PSUM group tail
                waits on the shortest chain. addt[t] enters the PSUM sum as
                EYE.T @ addt (fused path), freeing the DVE from the z-add.
                """
                y_ps = psp.tile([BPC, M], f32, tag="y", name="y")
                hot = min(max(t - 1, 0) // 4, NSTILE - 1)
                korder = [k for k in range(hot)] + [hot]
                if fused_relu6:
                    nc.tensor.matmul(y_ps[:], eye_sb[:], addt[t],
                                     start=True, stop=False)
                for i, k in enumerate(korder):
                    junk = wp.tile([128, 1], f32, tag="junk", name="junk")
                    cw = wp.tile([128, 1], f32, tag="cw", name="cw")
                    nc.vector.scalar_tensor_tensor(
                        out=junk.broadcast_to((128, M)),
                        in0=ph[k][:],
                        scalar=6.0 if fused_relu6 else 1.0,
                        in1=xb[t],
                        op0=Alu.min if fused_relu6 else Alu.bypass,
                        op1=Alu.mult,
                        accum_out=cw[:],
                    )
                    selcw = wp.tile([128, BPC], bf16, tag="selcw",
                                    name="selcw")
                    nc.vector.tensor_scalar(
                        out=selcw[:], in0=selw[t], scalar1=cw[:],
                        scalar2=None, op0=Alu.mult,
                    )
                    nc.tensor.matmul(
                        y_ps[:], selcw[:], xhw[k],
                        start=(i == 0) and not fused_relu6,
                        stop=(i == len(korder) - 1),
                    )
                return y_ps

            # t=0: no history yet -> z = addt[0], stored directly.
            if fused_relu6:
                nc.vector.tensor_copy(out=ph[0][0:BPC, :], in_=addt[0])
            else:
                nc.vector.tensor_scalar(
                    out=ph[0][0:BPC, :], in0=addt[0],
                    scalar1=0.0, scalar2=6.0, op0=Alu.max, op1=Alu.min,
                )
            for t in range(1, T + 1):
                y_ps = step_y(t)
                if t < T:
                    k, s4 = t // 4, t % 4
                    dst = ph[k][s4 * BPC:(s4 + 1) * BPC, :]
                    if fused_relu6:
                        nc.vector.tensor_copy(out=dst, in_=y_ps[:])
                    else:
                        zt = wp.tile([BPC, M], f32, tag="z", name="z")
                        nc.vector.tensor_add(out=zt[:], in0=y_ps[:],
                                             in1=addt[t])
                        nc.vector.tensor_scalar(
                            out=dst, in0=zt[:],
                            scalar1=0.0, scalar2=6.0,
                            op0=Alu.max, op1=Alu.min,
                        )
                else:
                    res = wp.tile([BPC, M], f32, tag="res", name="res")
                    if fused_relu6:
                        nc.vector.tensor_scalar(
                            out=res[:], in0=y_ps[:],
                            scalar1=0.0, scalar2=6.0,
                            op0=Alu.max, op1=Alu.min,
                        )
                    else:
                        z = wp.tile([BPC, M], f32, tag="z", name="z")
                        nc.vector.tensor_add(out=z[:], in0=y_ps[:],
                                             in1=addt[t])
                        nc.vector.tensor_scalar(
                            out=res[:], in0=z[:],
                            scalar1=0.0, scalar2=6.0,
                            op0=Alu.max, op1=Alu.min,
                        )
                    nc.sync.dma_start(out=out_d[:, :], in_=res[:])

    nc.finalize()
    return nc


def _get_program(dots_dtype_name, fused_relu6=True):
    key = (dots_dtype_name, fused_relu6)
    if key not in _COMPILED:
        import concourse.mybir as mybir
        _COMPILED[key] = _build_program(
            getattr(mybir.dt, dots_dtype_name), fused_relu6=fused_relu6
        )
    return _COMPILED[key]


def _prep_core_inputs(xs, x_query, q_terms, decay, learn, learn2, core,
                      np_dots, fused):
    """Host-side tensor prep for one core's batch slice (numpy only)."""
    g = 1.0 - decay
    bs = slice(core * BPC, (core + 1) * BPC)
    xs_c = xs[:, bs, :]          # [T, 32, M]
    xq_c = x_query[bs, :]        # [32, M]

    # XB[t] = x_t tiled over s4 (4x along partitions); XB[T] = x_query
    xb = np.empty((T + 1, 128, M), dtype=np_dots)
    for t in range(T):
        xb[t] = np.tile(xs_c[t], (4, 1))
    xb[T] = np.tile(xq_c, (4, 1))
    xb = np.ascontiguousarray(xb.transpose(1, 0, 2).reshape(128, (T + 1) * M))

    # XHW[k][s4*32+b, m] = learn * g^-(4k+s4+1) * xs[4k+s4, b, m]
    # DRAM layout [128, NSTILE*M]: partition p = s4*32+b, free = (k, m)
    s_idx = np.arange(T, dtype=np.float64)
    wneg = (learn * g ** (-(s_idx + 1.0))).astype(np.float32)  # [T]
    import ml_dtypes
    xhw4 = (xs_c.astype(np.float32) * wneg[:, None, None]).reshape(
        NSTILE, 4, BPC, M
    )  # [k, s4, b, m]
    xhw = xhw4.transpose(1, 2, 0, 3).reshape(128, NSTILE * M)
    xhw = xhw.astype(ml_dtypes.bfloat16)

    # SELW[t] = g^t * one-hot(b), partitions (s4, b); layout [128, (T+1)*32]
    eye = np.tile(np.eye(BPC, dtype=np.float32), (4, 1))  # [128, 32]
    gpow = (g ** np.arange(T + 1, dtype=np.float64)).astype(np.float32)
    selw = (gpow[:, None, None] * eye[None]).transpose(1, 0, 2).reshape(
        128, (T + 1) * BPC
    ).astype(np_dots if np_dots != np.float32 else np.float32)
    import ml_dtypes
    selw = selw.astype(ml_dtypes.bfloat16)

    # ADD[t] = learn2*x_t + g^t*(A_init@x_t);  ADD[16] = g^16*(A_init@x_q)
    addt = np.zeros((T + 1, BPC, M), dtype=np.float32)
    addt[:T] = learn2 * xs_c
    if q_terms is not None:
        q_c, qq_c = q_terms  # [T,32,M], [32,M] for this core slice
        addt[:T] += gpow[:T, None, None] * q_c
        addt[T] = gpow[T] * qq_c
    addt = addt.transpose(1, 0, 2).reshape(BPC, (T + 1) * M)  # [32, 17*256]
    if fused:
        addt = addt.astype(ml_dtypes.bfloat16)

    return {
        "xb": np.ascontiguousarray(xb),
        "xhw": np.ascontiguousarray(xhw),
        "selw": np.ascontiguousarray(selw),
        "addt": np.ascontiguousarray(addt),
        "eye": np.eye(BPC, dtype=ml_dtypes.bfloat16),
    }


def kernel(A_init, xs, x_query, decay, learn, learn2, _trace=False):
    from concourse.bass_utils import run_bass_kernel_spmd

    xs = np.asarray(xs, dtype=np.float32)
    x_query = np.asarray(x_query, dtype=np.float32)
    A_init = np.asarray(A_init, dtype=np.float32)
    decay_v = float(np.asarray(decay).reshape(-1)[0])
    learn_v = float(np.asarray(learn).reshape(-1)[0])
    learn2_v = float(np.asarray(learn2).reshape(-1)[0])

    dots_dtype_name = os.environ.get("KERNEL_DOTS_DTYPE", "bfloat16")
    if dots_dtype_name == "float32":
        np_dots = np.float32
    else:
        import ml_dtypes
        np_dots = ml_dtypes.bfloat16

    # relu6 folds into the history dots as min(.,6) only when the
    # pre-activations are provably nonnegative: A_init == 0 and all inputs
    # >= 0 (true for the problem spec). Otherwise build the general variant.
    a_zero = not A_init.any()
    fused = bool(a_zero and xs.min() >= 0.0 and x_query.min() >= 0.0)
    nc = _get_program(dots_dtype_name, fused_relu6=fused)

    in_maps = []
    for c in range(NCORES):
        q_terms = None
        if not a_zero:
            bs = slice(c * BPC, (c + 1) * BPC)
            a_c = A_init[bs]
            q_c = np.einsum("bij,tbj->tbi", a_c, xs[:, bs, :])
            qq_c = np.einsum("bij,bj->bi", a_c, x_query[bs])
            q_terms = (q_c, qq_c)
        in_maps.append(
            _prep_core_inputs(
                xs, x_query, q_terms, decay_v, learn_v, learn2_v, c,
                np_dots, fused
            )
        )

    res = run_bass_kernel_spmd(
        nc, in_maps, core_ids=list(range(NCORES)), trace=_trace
    )

    out = np.concatenate(
        [np.asarray(r["out"], dtype=np.float32) for r in res.results], axis=0
    )

    if _trace:
        return out, res
    return out



# revision 5
# speedup vs baseline: 1.0585x; 1.0585x over previous
"""Trainium2 Bass kernel for the Hebbian fast-weight memory module.

Reference computation (B=256 batches, T=16 steps, M=256):
    step t:  p2 = learn * relu6(learn2*x_t + A @ x_t)
             A  = (1-decay)*A + outer(x_t, p2)
    output:  relu6(A_final @ x_query)

Key identity (g = 1-decay, Phi_s = relu6(z_s), z_s = learn2*x_s + y_s):
    y_t[i] = g^t (A_init@x_t)[i] + sum_{s<t} g^{t-1-s} learn (Phi_s . x_t) x_s[i]
    out[i] = relu6(g^16 (A_init@x_q)[i] + sum_s g^{15-s} learn (Phi_s . x_q) x_s[i])
A is never materialized. For the fast path we additionally require
A_init == 0 and xs, x_query >= 0 (checked at runtime): then z >= 0 and
relu6(z) = min(z, 6) folds into the history dot products.

Fast-path layout per core (bpc=32 batches):
  partitions p = s4*32 + b  (s4 in [0,4), b in [0,32)), history step s = 4k+s4
  z-history lives IN PSUM: z_s at PSUM bank (s//4), partitions (s%4)*32+b,
  written directly by the accumulating matmuls (no per-step copy-out).
  Completed history tiles get one lazy ACT copy to SBUF (bf16), 3 total.

  Per step t: for each history tile k: a scalar_tensor_tensor on DVE/Pool
  computes cw_k[p] = sum_m min(z,6)*x_t (relu6 folded); ACT/DVE builds
  selcw_k = g^t*onehot(b)*cw_k; PE accumulates selcw_k.T @ xhw_k into
  z_t's PSUM slot. learn2*x_t enters via a tiny constant matmul (eyeL2).

Sharding: batch 256 -> 8 cores x 32 batches (pure data parallel).
The general path (A_init != 0 or negative inputs) uses the original
baseline program with host-precomputed additive terms.
"""

import os
import sys

for _p in ("/opt/pypackages", "/opt/trn_rl_repo"):
    if _p not in sys.path:
        sys.path.insert(0, _p)

import numpy as np

B, T, M = 256, 16, 256
NCORES = 8
BPC = B // NCORES  # 32 batches per core
NSTILE = 4         # history tiles; each holds 4 steps x 32 batches

_COMPILED = {}


def _dve_old(t, k):
    """True if the old-tile dot (t, k) runs on DVE instead of Pool."""
    return (t * 3 + k) % 4 == 0


def _build_program_fast():
    import concourse.bacc as bacc
    import concourse.mybir as mybir
    from concourse.tile import TileContext

    f32 = mybir.dt.float32
    bf16 = mybir.dt.bfloat16
    Alu = mybir.AluOpType
    Act = mybir.ActivationFunctionType

    nc = bacc.Bacc(target_bir_lowering=False)

    NQ = T + 1  # 16 steps + query
    xb_d = nc.dram_tensor("xb", [BPC, NQ * M], bf16, kind="ExternalInput")
    xhw_d = nc.dram_tensor("xhw", [128, NSTILE * M], bf16, kind="ExternalInput")
    selw_d = nc.dram_tensor("selw", [128, NQ * BPC], bf16, kind="ExternalInput")
    eyel2_d = nc.dram_tensor("eyel2", [BPC, BPC], bf16, kind="ExternalInput")
    out_d = nc.dram_tensor("out", [BPC, M], f32, kind="ExternalOutput")

    with TileContext(nc) as tc:
        with (
            tc.tile_pool(name="persist", bufs=1) as pp,
            tc.tile_pool(name="work", bufs=10) as wp,
            tc.tile_pool(name="psum", bufs=1, space="PSUM") as psp,
        ):
            xb_all = pp.tile([128, NQ * M], bf16, tag="xb", name="xb_sb")
            xb = [xb_all[:, t * M:(t + 1) * M] for t in range(NQ)]
            xhw_all = pp.tile([128, NSTILE * M], bf16, tag="xhw", name="xhw_sb")
            xhw = [xhw_all[:, k * M:(k + 1) * M] for k in range(NSTILE)]
            selw_all = pp.tile([128, NQ * BPC], bf16, tag="selw", name="selw_sb")
            selw = [selw_all[:, t * BPC:(t + 1) * BPC] for t in range(NQ)]
            eyel2 = pp.tile([BPC, BPC], bf16, tag="eyel2", name="eyel2_sb")
            # SBUF copies of completed history tiles (k = 0..2; k=3 stays in
            # PSUM through the query step)
            ph = [pp.tile([128, M], bf16, tag=f"ph{k}", name=f"ph{k}")
                  for k in range(NSTILE - 1)]
            junk_d = pp.tile([128, M], bf16, tag="junkd", name="junk_dve")
            junk_p = pp.tile([128, M], bf16, tag="junkp", name="junk_pool")
            # z history in PSUM: bank k holds steps 4k..4k+3 (k=4: query y)
            zb = [psp.tile([128, M], f32, tag=f"zb{k}", name=f"zb{k}")
                  for k in range(NSTILE + 1)]

            # --- input DMAs, split across queues; earliest-needed first ---
            nc.sync.dma_start(out=eyel2[:], in_=eyel2_d[:, :])
            nc.scalar.dma_start(out=xb_all[0:BPC, :], in_=xb_d[:, :])
            nc.sync.dma_start(out=xhw_all[:], in_=xhw_d[:, :])
            nc.gpsimd.dma_start(out=selw_all[:], in_=selw_d[:, :])
            # replicate x over the 4 s4 partition blocks (SBUF->SBUF)
            for r in range(1, 4):
                eng = (nc.scalar, nc.sync, nc.gpsimd)[r - 1]
                eng.dma_start(
                    out=xb_all[r * BPC:(r + 1) * BPC, :],
                    in_=xb_all[0:BPC, :],
                )

            def zslot(s):
                q = s % 4
                return zb[s // 4][q * BPC:(q + 1) * BPC, :]

            # t=0: z_0 = learn2 * x_0 via the constant eyeL2 matmul
            nc.tensor.matmul(zslot(0), eyel2[:], xb[0][0:BPC, :],
                             start=True, stop=True, tile_position=(0, 0))

            for t in range(1, NQ):
                is_q = (t == T)
                hot = (t - 1) // 4
                nrow = ((t - 1) % 4) + 1  # valid row-blocks in hot tile
                cws = []
                for k in range(hot + 1):
                    cw = wp.tile([128, 1], f32, tag="cw", name="cw")
                    if k == hot:
                        # hot tile: read z straight from PSUM (f32)
                        np_ = nrow * BPC
                        nc.vector.scalar_tensor_tensor(
                            out=junk_d[0:np_, :],
                            in0=zb[hot][0:np_, :],
                            scalar=6.0,
                            in1=xb[t][0:np_, :],
                            op0=Alu.min, op1=Alu.mult,
                            accum_out=cw[0:np_, :],
                        )
                    else:
                        nc.vector.scalar_tensor_tensor(
                            out=junk_d[:],
                            in0=ph[k][:],
                            scalar=6.0,
                            in1=xb[t],
                            op0=Alu.min, op1=Alu.mult,
                            accum_out=cw[:],
                        )
                    cws.append(cw)

                # selcw_k = selw[t] * cw_k  (per-partition scale)
                selcws = []
                for k in range(hot + 1):
                    np_ = nrow * BPC if k == hot else 128
                    selcw = wp.tile([128, BPC], bf16, tag="selcw",
                                    name="selcw")
                    nc.scalar.activation(
                        out=selcw[0:np_, :], in_=selw[t][0:np_, :],
                        func=Act.Copy, scale=cws[k][0:np_, :],
                    )
                    selcws.append(selcw)

                # accumulate z_t (or the query y) in its PSUM slot
                dst = zslot(t) if not is_q else zb[NSTILE][0:BPC, :]
                tpos = (0, (t % 4) * BPC) if not is_q else (0, 0)
                if not is_q:
                    nc.tensor.matmul(dst, eyel2[:], xb[t][0:BPC, :],
                                     start=True, stop=False,
                                     tile_position=tpos)
                for k in range(hot + 1):
                    np_ = nrow * BPC if k == hot else 128
                    nc.tensor.matmul(
                        dst, selcws[k][0:np_, :], xhw[k][0:np_, :],
                        start=(is_q and k == 0),
                        stop=(k == hot),
                        tile_position=tpos,
                    )

                # lazy SBUF copy of a tile the moment it completes
                # (tile k complete once z_{4k+3} formed at step t=4k+3)
                if t % 4 == 3 and t // 4 < NSTILE - 1:
                    nc.scalar.activation(
                        out=ph[t // 4][:], in_=zb[t // 4][:],
                        func=Act.Copy,
                    )

            res = wp.tile([BPC, M], f32, tag="res", name="res")
            nc.vector.tensor_scalar(
                out=res[:], in0=zb[NSTILE][0:BPC, :],
                scalar1=0.0, scalar2=6.0, op0=Alu.max, op1=Alu.min,
            )
            nc.sync.dma_start(out=out_d[:, :], in_=res[:])

    nc.finalize()
    return nc


def _build_program_general(dots_dtype):
    """Baseline program: general path (A_init != 0 or negative inputs)."""
    import concourse.bacc as bacc
    import concourse.mybir as mybir
    from concourse.tile import TileContext

    f32 = mybir.dt.float32
    bf16 = mybir.dt.bfloat16
    Alu = mybir.AluOpType

    nc = bacc.Bacc(target_bir_lowering=False)

    xb_d = nc.dram_tensor("xb", [128, (T + 1) * M], dots_dtype,
                          kind="ExternalInput")
    xhw_d = nc.dram_tensor("xhw", [128, NSTILE * M], bf16, kind="ExternalInput")
    selw_d = nc.dram_tensor("selw", [128, (T + 1) * BPC], bf16,
                            kind="ExternalInput")
    add_d = nc.dram_tensor("addt", [BPC, (T + 1) * M], f32,
                           kind="ExternalInput")
    out_d = nc.dram_tensor("out", [BPC, M], f32, kind="ExternalOutput")

    with TileContext(nc) as tc:
        with (
            tc.tile_pool(name="persist", bufs=1) as pp,
            tc.tile_pool(name="work", bufs=8) as wp,
            tc.tile_pool(name="psum", bufs=6, space="PSUM") as psp,
        ):
            xb_all = pp.tile([128, (T + 1) * M], dots_dtype, tag="xb",
                             name="xb_sb")
            xb = [xb_all[:, t * M:(t + 1) * M] for t in range(T + 1)]
            xhw_all = pp.tile([128, NSTILE * M], bf16, tag="xhw", name="xhw_sb")
            xhw = [xhw_all[:, k * M:(k + 1) * M] for k in range(NSTILE)]
            selw_all = pp.tile([128, (T + 1) * BPC], bf16, tag="selw",
                               name="selw_sb")
            selw = [selw_all[:, t * BPC:(t + 1) * BPC] for t in range(T + 1)]
            addt_all = pp.tile([BPC, (T + 1) * M], f32, tag="addt",
                               name="addt_sb")
            addt = [addt_all[:, t * M:(t + 1) * M] for t in range(T + 1)]
            ph = [pp.tile([128, M], dots_dtype, tag=f"ph{k}", name=f"ph{k}")
                  for k in range(NSTILE)]

            XB_SPLIT = 5 * M
            nc.scalar.dma_start(out=xb_all[:, :XB_SPLIT],
                                in_=xb_d[:, :XB_SPLIT])
            nc.scalar.dma_start(out=addt_all[:], in_=add_d[:, :])
            nc.sync.dma_start(out=selw_all[:], in_=selw_d[:, :])
            nc.sync.dma_start(out=xhw_all[:], in_=xhw_d[:, :])
            nc.sync.dma_start(out=xb_all[:, XB_SPLIT:], in_=xb_d[:, XB_SPLIT:])

            for k in range(NSTILE):
                nc.vector.memset(ph[k][:], 0.0)

            def step_y(t):
                y_ps = psp.tile([BPC, M], f32, tag="y", name="y")
                hot = min(max(t - 1, 0) // 4, NSTILE - 1)
                korder = list(range(hot + 1))
                for i, k in enumerate(korder):
                    junk = wp.tile([128, 1], f32, tag="junk", name="junk")
                    cw = wp.tile([128, 1], f32, tag="cw", name="cw")
                    nc.vector.scalar_tensor_tensor(
                        out=junk.broadcast_to((128, M)),
                        in0=ph[k][:],
                        scalar=1.0,
                        in1=xb[t],
                        op0=Alu.bypass,
                        op1=Alu.mult,
                        accum_out=cw[:],
                    )
                    selcw = wp.tile([128, BPC], bf16, tag="selcw",
                                    name="selcw")
                    nc.vector.tensor_scalar(
                        out=selcw[:], in0=selw[t], scalar1=cw[:],
                        scalar2=None, op0=Alu.mult,
                    )
                    nc.tensor.matmul(
                        y_ps[:], selcw[:], xhw[k],
                        start=(i == 0),
                        stop=(i == len(korder) - 1),
                    )
                return y_ps

            nc.vector.tensor_scalar(
                out=ph[0][0:BPC, :], in0=addt[0],
                scalar1=0.0, scalar2=6.0, op0=Alu.max, op1=Alu.min,
            )
            for t in range(1, T + 1):
                y_ps = step_y(t)
                if t < T:
                    k, s4 = t // 4, t % 4
                    dst = ph[k][s4 * BPC:(s4 + 1) * BPC, :]
                    zt = wp.tile([BPC, M], f32, tag="z", name="z")
                    nc.vector.tensor_add(out=zt[:], in0=y_ps[:],
                                         in1=addt[t])
                    nc.vector.tensor_scalar(
                        out=dst, in0=zt[:],
                        scalar1=0.0, scalar2=6.0,
                        op0=Alu.max, op1=Alu.min,
                    )
                else:
                    res = wp.tile([BPC, M], f32, tag="res", name="res")
                    z = wp.tile([BPC, M], f32, tag="z", name="z")
                    nc.vector.tensor_add(out=z[:], in0=y_ps[:],
                                         in1=addt[t])
                    nc.vector.tensor_scalar(
                        out=res[:], in0=z[:],
                        scalar1=0.0, scalar2=6.0,
                        op0=Alu.max, op1=Alu.min,
                    )
                    nc.sync.dma_start(out=out_d[:, :], in_=res[:])

    nc.finalize()
    return nc


def _get_program(fast):
    key = ("fast",) if fast else ("general",)
    if key not in _COMPILED:
        if fast:
            _COMPILED[key] = _build_program_fast()
        else:
            import concourse.mybir as mybir
            _COMPILED[key] = _build_program_general(mybir.dt.float32)
    return _COMPILED[key]


def _prep_fast(xs, x_query, decay, learn, learn2, core):
    """Host-side tensor prep for one core's batch slice (fast path)."""
    import ml_dtypes
    g = 1.0 - decay
    bs = slice(core * BPC, (core + 1) * BPC)
    xs_c = xs[:, bs, :]          # [T, 32, M]
    xq_c = x_query[bs, :]        # [32, M]

    # xb: [32, (T+1)*M], query in slot T; replicated on-chip
    xb = np.concatenate([xs_c, xq_c[None]], axis=0)  # [17, 32, M]
    xb = np.ascontiguousarray(
        xb.transpose(1, 0, 2).reshape(BPC, (T + 1) * M)
    ).astype(ml_dtypes.bfloat16)

    # xhw[k][s4*32+b, m] = learn * g^-(4k+s4+1) * xs[4k+s4, b, m]
    s_idx = np.arange(T, dtype=np.float64)
    wneg = (learn * g ** (-(s_idx + 1.0))).astype(np.float32)
    xhw4 = (xs_c.astype(np.float32) * wneg[:, None, None]).reshape(
        NSTILE, 4, BPC, M
    )
    xhw = xhw4.transpose(1, 2, 0, 3).reshape(128, NSTILE * M)
    xhw = xhw.astype(ml_dtypes.bfloat16)

    # selw[t] = g^t * one-hot(b); partitions (s4, b)
    eye = np.tile(np.eye(BPC, dtype=np.float32), (4, 1))  # [128, 32]
    gpow = (g ** np.arange(T + 1, dtype=np.float64)).astype(np.float32)
    selw = (gpow[:, None, None] * eye[None]).transpose(1, 0, 2).reshape(
        128, (T + 1) * BPC
    ).astype(ml_dtypes.bfloat16)

    eyel2 = (learn2 * np.eye(BPC, dtype=np.float32)).astype(ml_dtypes.bfloat16)

    return {
        "xb": np.ascontiguousarray(xb),
        "xhw": np.ascontiguousarray(xhw),
        "selw": np.ascontiguousarray(selw),
        "eyel2": np.ascontiguousarray(eyel2),
    }


def _prep_general(xs, x_query, A_init, decay, learn, learn2, core):
    import ml_dtypes
    g = 1.0 - decay
    bs = slice(core * BPC, (core + 1) * BPC)
    xs_c = xs[:, bs, :]
    xq_c = x_query[bs, :]
    a_c = A_init[bs]

    xb = np.empty((T + 1, 128, M), dtype=np.float32)
    for t in range(T):
        xb[t] = np.tile(xs_c[t], (4, 1))
    xb[T] = np.tile(xq_c, (4, 1))
    xb = np.ascontiguousarray(xb.transpose(1, 0, 2).reshape(128, (T + 1) * M))

    s_idx = np.arange(T, dtype=np.float64)
    wneg = (learn * g ** (-(s_idx + 1.0))).astype(np.float32)
    xhw4 = (xs_c.astype(np.float32) * wneg[:, None, None]).reshape(
        NSTILE, 4, BPC, M
    )
    xhw = xhw4.transpose(1, 2, 0, 3).reshape(128, NSTILE * M)
    xhw = xhw.astype(ml_dtypes.bfloat16)

    eye = np.tile(np.eye(BPC, dtype=np.float32), (4, 1))
    gpow = (g ** np.arange(T + 1, dtype=np.float64)).astype(np.float32)
    selw = (gpow[:, None, None] * eye[None]).transpose(1, 0, 2).reshape(
        128, (T + 1) * BPC
    ).astype(ml_dtypes.bfloat16)

    q_c = np.einsum("bij,tbj->tbi", a_c, xs_c)
    qq_c = np.einsum("bij,bj->bi", a_c, xq_c)
    addt = np.zeros((T + 1, BPC, M), dtype=np.float32)
    addt[:T] = learn2 * xs_c
    addt[:T] += gpow[:T, None, None] * q_c
    addt[T] = gpow[T] * qq_c
    addt = addt.transpose(1, 0, 2).reshape(BPC, (T + 1) * M)

    return {
        "xb": np.ascontiguousarray(xb),
        "xhw": np.ascontiguousarray(xhw),
        "selw": np.ascontiguousarray(selw),
        "addt": np.ascontiguousarray(addt),
    }


def kernel(A_init, xs, x_query, decay, learn, learn2, _trace=False):
    from concourse.bass_utils import run_bass_kernel_spmd

    xs = np.asarray(xs, dtype=np.float32)
    x_query = np.asarray(x_query, dtype=np.float32)
    A_init = np.asarray(A_init, dtype=np.float32)
    decay_v = float(np.asarray(decay).reshape(-1)[0])
    learn_v = float(np.asarray(learn).reshape(-1)[0])
    learn2_v = float(np.asarray(learn2).reshape(-1)[0])

    # The relu6 -> min(.,6) fold inside the history dots requires provably
    # nonnegative pre-activations: A_init == 0 and all inputs >= 0.
    a_zero = not A_init.any()
    fast = bool(a_zero and xs.min() >= 0.0 and x_query.min() >= 0.0)
    nc = _get_program(fast)

    in_maps = []
    for c in range(NCORES):
        if fast:
            in_maps.append(
                _prep_fast(xs, x_query, decay_v, learn_v, learn2_v, c)
            )
        else:
            in_maps.append(
                _prep_general(xs, x_query, A_init, decay_v, learn_v,
                              learn2_v, c)
            )

    res = run_bass_kernel_spmd(
        nc, in_maps, core_ids=list(range(NCORES)), trace=_trace
    )

    out = np.concatenate(
        [np.asarray(r["out"], dtype=np.float32) for r in res.results], axis=0
    )

    if _trace:
        return out, res
    return out


# revision 6
# speedup vs baseline: 1.1025x; 1.0416x over previous
"""Trainium2 Bass kernel for the Hebbian fast-weight memory module.

Reference computation (B=256 batches, T=16 steps, M=256):
    step t:  p2 = learn * relu6(learn2*x_t + A @ x_t)
             A  = (1-decay)*A + outer(x_t, p2)
    output:  relu6(A_final @ x_query)

Key identity (g = 1-decay, Phi_s = relu6(z_s), z_s = learn2*x_s + y_s):
    y_t[i] = g^t (A_init@x_t)[i] + sum_{s<t} g^{t-1-s} learn (Phi_s . x_t) x_s[i]
    out[i] = relu6(g^16 (A_init@x_q)[i] + sum_s g^{15-s} learn (Phi_s . x_q) x_s[i])
A is never materialized. For the fast path we additionally require
A_init == 0 and xs, x_query >= 0 (checked at runtime): then z >= 0 and
relu6(z) = min(z, 6) folds into the history dot products.

Fast-path layout per core (bpc=32 batches):
  partitions p = s4*32 + b  (s4 in [0,4), b in [0,32)), history step s = 4k+s4
  z-history lives IN PSUM: z_s at PSUM bank (s//4), partitions (s%4)*32+b,
  written directly by the accumulating matmuls (no per-step copy-out).
  Completed history tiles get one lazy ACT copy to SBUF (bf16), 3 total.

  Per step t: for each history tile k: a scalar_tensor_tensor on DVE/Pool
  computes cw_k[p] = sum_m min(z,6)*x_t (relu6 folded); ACT/DVE builds
  selcw_k = g^t*onehot(b)*cw_k; PE accumulates selcw_k.T @ xhw_k into
  z_t's PSUM slot. learn2*x_t enters via a tiny constant matmul (eyeL2).

Sharding: batch 256 -> 8 cores x 32 batches (pure data parallel).
The general path (A_init != 0 or negative inputs) uses the original
baseline program with host-precomputed additive terms.
"""

import os
import sys

for _p in ("/opt/pypackages", "/opt/trn_rl_repo"):
    if _p not in sys.path:
        sys.path.insert(0, _p)

import numpy as np

B, T, M = 256, 16, 256
NCORES = 8
BPC = B // NCORES  # 32 batches per core
NSTILE = 4         # history tiles; each holds 4 steps x 32 batches

_COMPILED = {}


def _dve_old(t, k):
    """True if the old-tile dot (t, k) runs on DVE instead of Pool."""
    return (t * 3 + k) % 4 == 0


def _build_program_fast():
    import concourse.bacc as bacc
    import concourse.mybir as mybir
    from concourse.tile import TileContext

    f32 = mybir.dt.float32
    bf16 = mybir.dt.bfloat16
    Alu = mybir.AluOpType
    Act = mybir.ActivationFunctionType

    nc = bacc.Bacc(target_bir_lowering=False)

    NQ = T + 1  # 16 steps + query
    xb_d = nc.dram_tensor("xb", [BPC, NQ * M], bf16, kind="ExternalInput")
    xhw_d = nc.dram_tensor("xhw", [128, NSTILE * M], bf16, kind="ExternalInput")
    selw_d = nc.dram_tensor("selw", [128, NQ * BPC], bf16, kind="ExternalInput")
    eyel2_d = nc.dram_tensor("eyel2", [BPC, BPC], bf16, kind="ExternalInput")
    out_d = nc.dram_tensor("out", [BPC, M], f32, kind="ExternalOutput")

    with TileContext(nc) as tc:
        with (
            tc.tile_pool(name="persist", bufs=1) as pp,
            tc.tile_pool(name="work", bufs=10) as wp,
            tc.tile_pool(name="psum", bufs=1, space="PSUM") as psp,
        ):
            xb_all = pp.tile([128, NQ * M], bf16, tag="xb", name="xb_sb")
            xb = [xb_all[:, t * M:(t + 1) * M] for t in range(NQ)]
            xhw_all = pp.tile([128, NSTILE * M], bf16, tag="xhw", name="xhw_sb")
            xhw = [xhw_all[:, k * M:(k + 1) * M] for k in range(NSTILE)]
            selw_all = pp.tile([128, NQ * BPC], bf16, tag="selw", name="selw_sb")
            selw = [selw_all[:, t * BPC:(t + 1) * BPC] for t in range(NQ)]
            eyel2 = pp.tile([BPC, BPC], bf16, tag="eyel2", name="eyel2_sb")
            # SBUF copies of completed history tiles (k = 0..2; k=3 stays in
            # PSUM through the query step)
            ph = [pp.tile([128, M], bf16, tag=f"ph{k}", name=f"ph{k}")
                  for k in range(NSTILE - 1)]
            junk_d = pp.tile([128, M], bf16, tag="junkd", name="junk_dve")
            junk_p = pp.tile([128, M], bf16, tag="junkp", name="junk_pool")
            # z history in PSUM: bank k holds steps 4k..4k+3 (k=4: query y)
            zb = [psp.tile([128, M], f32, tag=f"zb{k}", name=f"zb{k}")
                  for k in range(NSTILE + 1)]

            # --- input DMAs, split across queues; earliest-needed first ---
            nc.sync.dma_start(out=eyel2[:], in_=eyel2_d[:, :])
            # xb: chunked by step range so early steps aren't gated on the
            # full load; each chunk is replicated to the 4 s4 blocks right
            # after it lands.
            CH = [(0, 3 * M), (3 * M, 9 * M), (9 * M, NQ * M)]
            for lo, hi in CH:
                nc.scalar.dma_start(out=xb_all[0:BPC, lo:hi],
                                    in_=xb_d[:, lo:hi])
            nc.sync.dma_start(out=xhw_all[:], in_=xhw_d[:, :])
            nc.gpsimd.dma_start(out=selw_all[:], in_=selw_d[:, :])
            for lo, hi in CH:
                for r in range(1, 4):
                    eng = (nc.scalar, nc.sync, nc.gpsimd)[r - 1]
                    eng.dma_start(
                        out=xb_all[r * BPC:(r + 1) * BPC, lo:hi],
                        in_=xb_all[0:BPC, lo:hi],
                    )

            def zslot(s):
                q = s % 4
                return zb[s // 4][q * BPC:(q + 1) * BPC, :]

            # t=0: z_0 = learn2 * x_0 via the constant eyeL2 matmul
            nc.tensor.matmul(zslot(0), eyel2[:], xb[0][0:BPC, :],
                             start=True, stop=True, tile_position=(0, 0))

            for t in range(1, NQ):
                is_q = (t == T)
                hot = (t - 1) // 4
                nrow = ((t - 1) % 4) + 1  # valid row-blocks in hot tile
                cws = []
                for k in range(hot + 1):
                    cw = wp.tile([128, 1], f32, tag="cw", name="cw")
                    if k == hot:
                        # hot tile: read z straight from PSUM (f32)
                        np_ = nrow * BPC
                        nc.vector.scalar_tensor_tensor(
                            out=junk_d[0:np_, :],
                            in0=zb[hot][0:np_, :],
                            scalar=6.0,
                            in1=xb[t][0:np_, :],
                            op0=Alu.min, op1=Alu.mult,
                            accum_out=cw[0:np_, :],
                        )
                    else:
                        nc.vector.scalar_tensor_tensor(
                            out=junk_d[:],
                            in0=ph[k][:],
                            scalar=6.0,
                            in1=xb[t],
                            op0=Alu.min, op1=Alu.mult,
                            accum_out=cw[:],
                        )
                    cws.append(cw)

                # selcw_k = selw[t] * cw_k  (per-partition scale)
                selcws = []
                for k in range(hot + 1):
                    np_ = nrow * BPC if k == hot else 128
                    selcw = wp.tile([128, BPC], bf16, tag="selcw",
                                    name="selcw")
                    if k == hot:
                        nc.vector.tensor_scalar(
                            out=selcw[0:np_, :], in0=selw[t][0:np_, :],
                            scalar1=cws[k][0:np_, :], scalar2=None,
                            op0=Alu.mult,
                        )
                    else:
                        nc.scalar.activation(
                            out=selcw[0:np_, :], in_=selw[t][0:np_, :],
                            func=Act.Copy, scale=cws[k][0:np_, :],
                        )
                    selcws.append(selcw)

                # accumulate z_t (or the query y) in its PSUM slot
                dst = zslot(t) if not is_q else zb[NSTILE][0:BPC, :]
                tpos = (0, (t % 4) * BPC) if not is_q else (0, 0)
                if not is_q:
                    nc.tensor.matmul(dst, eyel2[:], xb[t][0:BPC, :],
                                     start=True, stop=False,
                                     tile_position=tpos)
                for k in range(hot + 1):
                    np_ = nrow * BPC if k == hot else 128
                    nc.tensor.matmul(
                        dst, selcws[k][0:np_, :], xhw[k][0:np_, :],
                        start=(is_q and k == 0),
                        stop=(k == hot),
                        tile_position=tpos,
                    )

                # lazy SBUF copy of a tile the moment it completes
                # (tile k complete once z_{4k+3} formed at step t=4k+3)
                if t % 4 == 3 and t // 4 < NSTILE - 1:
                    nc.scalar.activation(
                        out=ph[t // 4][:], in_=zb[t // 4][:],
                        func=Act.Copy,
                    )

            res = wp.tile([BPC, M], f32, tag="res", name="res")
            nc.vector.tensor_scalar(
                out=res[:], in0=zb[NSTILE][0:BPC, :],
                scalar1=0.0, scalar2=6.0, op0=Alu.max, op1=Alu.min,
            )
            nc.sync.dma_start(out=out_d[:, :], in_=res[:])

    nc.finalize()
    return nc


def _build_program_general(dots_dtype):
    """Baseline program: general path (A_init != 0 or negative inputs)."""
    import concourse.bacc as bacc
    import concourse.mybir as mybir
    from concourse.tile import TileContext

    f32 = mybir.dt.float32
    bf16 = mybir.dt.bfloat16
    Alu = mybir.AluOpType

    nc = bacc.Bacc(target_bir_lowering=False)

    xb_d = nc.dram_tensor("xb", [128, (T + 1) * M], dots_dtype,
                          kind="ExternalInput")
    xhw_d = nc.dram_tensor("xhw", [128, NSTILE * M], bf16, kind="ExternalInput")
    selw_d = nc.dram_tensor("selw", [128, (T + 1) * BPC], bf16,
                            kind="ExternalInput")
    add_d = nc.dram_tensor("addt", [BPC, (T + 1) * M], f32,
                           kind="ExternalInput")
    out_d = nc.dram_tensor("out", [BPC, M], f32, kind="ExternalOutput")

    with TileContext(nc) as tc:
        with (
            tc.tile_pool(name="persist", bufs=1) as pp,
            tc.tile_pool(name="work", bufs=8) as wp,
            tc.tile_pool(name="psum", bufs=6, space="PSUM") as psp,
        ):
            xb_all = pp.tile([128, (T + 1) * M], dots_dtype, tag="xb",
                             name="xb_sb")
            xb = [xb_all[:, t * M:(t + 1) * M] for t in range(T + 1)]
            xhw_all = pp.tile([128, NSTILE * M], bf16, tag="xhw", name="xhw_sb")
            xhw = [xhw_all[:, k * M:(k + 1) * M] for k in range(NSTILE)]
            selw_all = pp.tile([128, (T + 1) * BPC], bf16, tag="selw",
                               name="selw_sb")
            selw = [selw_all[:, t * BPC:(t + 1) * BPC] for t in range(T + 1)]
            addt_all = pp.tile([BPC, (T + 1) * M], f32, tag="addt",
                               name="addt_sb")
            addt = [addt_all[:, t * M:(t + 1) * M] for t in range(T + 1)]
            ph = [pp.tile([128, M], dots_dtype, tag=f"ph{k}", name=f"ph{k}")
                  for k in range(NSTILE)]

            XB_SPLIT = 5 * M
            nc.scalar.dma_start(out=xb_all[:, :XB_SPLIT],
                                in_=xb_d[:, :XB_SPLIT])
            nc.scalar.dma_start(out=addt_all[:], in_=add_d[:, :])
            nc.sync.dma_start(out=selw_all[:], in_=selw_d[:, :])
            nc.sync.dma_start(out=xhw_all[:], in_=xhw_d[:, :])
            nc.sync.dma_start(out=xb_all[:, XB_SPLIT:], in_=xb_d[:, XB_SPLIT:])

            for k in range(NSTILE):
                nc.vector.memset(ph[k][:], 0.0)

            def step_y(t):
                y_ps = psp.tile([BPC, M], f32, tag="y", name="y")
                hot = min(max(t - 1, 0) // 4, NSTILE - 1)
                korder = list(range(hot + 1))
                for i, k in enumerate(korder):
                    junk = wp.tile([128, 1], f32, tag="junk", name="junk")
                    cw = wp.tile([128, 1], f32, tag="cw", name="cw")
                    nc.vector.scalar_tensor_tensor(
                        out=junk.broadcast_to((128, M)),
                        in0=ph[k][:],
                        scalar=1.0,
                        in1=xb[t],
                        op0=Alu.bypass,
                        op1=Alu.mult,
                        accum_out=cw[:],
                    )
                    selcw = wp.tile([128, BPC], bf16, tag="selcw",
                                    name="selcw")
                    nc.vector.tensor_scalar(
                        out=selcw[:], in0=selw[t], scalar1=cw[:],
                        scalar2=None, op0=Alu.mult,
                    )
                    nc.tensor.matmul(
                        y_ps[:], selcw[:], xhw[k],
                        start=(i == 0),
                        stop=(i == len(korder) - 1),
                    )
                return y_ps

            nc.vector.tensor_scalar(
                out=ph[0][0:BPC, :], in0=addt[0],
                scalar1=0.0, scalar2=6.0, op0=Alu.max, op1=Alu.min,
            )
            for t in range(1, T + 1):
                y_ps = step_y(t)
                if t < T:
                    k, s4 = t // 4, t % 4
                    dst = ph[k][s4 * BPC:(s4 + 1) * BPC, :]
                    zt = wp.tile([BPC, M], f32, tag="z", name="z")
                    nc.vector.tensor_add(out=zt[:], in0=y_ps[:],
                                         in1=addt[t])
                    nc.vector.tensor_scalar(
                        out=dst, in0=zt[:],
                        scalar1=0.0, scalar2=6.0,
                        op0=Alu.max, op1=Alu.min,
                    )
                else:
                    res = wp.tile([BPC, M], f32, tag="res", name="res")
                    z = wp.tile([BPC, M], f32, tag="z", name="z")
                    nc.vector.tensor_add(out=z[:], in0=y_ps[:],
                                         in1=addt[t])
                    nc.vector.tensor_scalar(
                        out=res[:], in0=z[:],
                        scalar1=0.0, scalar2=6.0,
                        op0=Alu.max, op1=Alu.min,
                    )
                    nc.sync.dma_start(out=out_d[:, :], in_=res[:])

    nc.finalize()
    return nc


def _get_program(fast):
    key = ("fast",) if fast else ("general",)
    if key not in _COMPILED:
        if fast:
            _COMPILED[key] = _build_program_fast()
        else:
            import concourse.mybir as mybir
            _COMPILED[key] = _build_program_general(mybir.dt.float32)
    return _COMPILED[key]


def _prep_fast(xs, x_query, decay, learn, learn2, core):
    """Host-side tensor prep for one core's batch slice (fast path)."""
    import ml_dtypes
    g = 1.0 - decay
    bs = slice(core * BPC, (core + 1) * BPC)
    xs_c = xs[:, bs, :]          # [T, 32, M]
    xq_c = x_query[bs, :]        # [32, M]

    # xb: [32, (T+1)*M], query in slot T; replicated on-chip
    xb = np.concatenate([xs_c, xq_c[None]], axis=0)  # [17, 32, M]
    xb = np.ascontiguousarray(
        xb.transpose(1, 0, 2).reshape(BPC, (T + 1) * M)
    ).astype(ml_dtypes.bfloat16)

    # xhw[k][s4*32+b, m] = learn * g^-(4k+s4+1) * xs[4k+s4, b, m]
    s_idx = np.arange(T, dtype=np.float64)
    wneg = (learn * g ** (-(s_idx + 1.0))).astype(np.float32)
    xhw4 = (xs_c.astype(np.float32) * wneg[:, None, None]).reshape(
        NSTILE, 4, BPC, M
    )
    xhw = xhw4.transpose(1, 2, 0, 3).reshape(128, NSTILE * M)
    xhw = xhw.astype(ml_dtypes.bfloat16)

    # selw[t] = g^t * one-hot(b); partitions (s4, b)
    eye = np.tile(np.eye(BPC, dtype=np.float32), (4, 1))  # [128, 32]
    gpow = (g ** np.arange(T + 1, dtype=np.float64)).astype(np.float32)
    selw = (gpow[:, None, None] * eye[None]).transpose(1, 0, 2).reshape(
        128, (T + 1) * BPC
    ).astype(ml_dtypes.bfloat16)

    eyel2 = (learn2 * np.eye(BPC, dtype=np.float32)).astype(ml_dtypes.bfloat16)

    return {
        "xb": np.ascontiguousarray(xb),
        "xhw": np.ascontiguousarray(xhw),
        "selw": np.ascontiguousarray(selw),
        "eyel2": np.ascontiguousarray(eyel2),
    }


def _prep_general(xs, x_query, A_init, decay, learn, learn2, core):
    import ml_dtypes
    g = 1.0 - decay
    bs = slice(core * BPC, (core + 1) * BPC)
    xs_c = xs[:, bs, :]
    xq_c = x_query[bs, :]
    a_c = A_init[bs]

    xb = np.empty((T + 1, 128, M), dtype=np.float32)
    for t in range(T):
        xb[t] = np.tile(xs_c[t], (4, 1))
    xb[T] = np.tile(xq_c, (4, 1))
    xb = np.ascontiguousarray(xb.transpose(1, 0, 2).reshape(128, (T + 1) * M))

    s_idx = np.arange(T, dtype=np.float64)
    wneg = (learn * g ** (-(s_idx + 1.0))).astype(np.float32)
    xhw4 = (xs_c.astype(np.float32) * wneg[:, None, None]).reshape(
        NSTILE, 4, BPC, M
    )
    xhw = xhw4.transpose(1, 2, 0, 3).reshape(128, NSTILE * M)
    xhw = xhw.astype(ml_dtypes.bfloat16)

    eye = np.tile(np.eye(BPC, dtype=np.float32), (4, 1))
    gpow = (g ** np.arange(T + 1, dtype=np.float64)).astype(np.float32)
    selw = (gpow[:, None, None] * eye[None]).transpose(1, 0, 2).reshape(
        128, (T + 1) * BPC
    ).astype(ml_dtypes.bfloat16)

    q_c = np.einsum("bij,tbj->tbi", a_c, xs_c)
    qq_c = np.einsum("bij,bj->bi", a_c, xq_c)
    addt = np.zeros((T + 1, BPC, M), dtype=np.float32)
    addt[:T] = learn2 * xs_c
    addt[:T] += gpow[:T, None, None] * q_c
    addt[T] = gpow[T] * qq_c
    addt = addt.transpose(1, 0, 2).reshape(BPC, (T + 1) * M)

    return {
        "xb": np.ascontiguousarray(xb),
        "xhw": np.ascontiguousarray(xhw),
        "selw": np.ascontiguousarray(selw),
        "addt": np.ascontiguousarray(addt),
    }


def kernel(A_init, xs, x_query, decay, learn, learn2, _trace=False):
    from concourse.bass_utils import run_bass_kernel_spmd

    xs = np.asarray(xs, dtype=np.float32)
    x_query = np.asarray(x_query, dtype=np.float32)
    A_init = np.asarray(A_init, dtype=np.float32)
    decay_v = float(np.asarray(decay).reshape(-1)[0])
    learn_v = float(np.asarray(learn).reshape(-1)[0])
    learn2_v = float(np.asarray(learn2).reshape(-1)[0])

    # The relu6 -> min(.,6) fold inside the history dots requires provably
    # nonnegative pre-activations: A_init == 0 and all inputs >= 0.
    a_zero = not A_init.any()
    fast = bool(a_zero and xs.min() >= 0.0 and x_query.min() >= 0.0)
    nc = _get_program(fast)

    in_maps = []
    for c in range(NCORES):
        if fast:
            in_maps.append(
                _prep_fast(xs, x_query, decay_v, learn_v, learn2_v, c)
            )
        else:
            in_maps.append(
                _prep_general(xs, x_query, A_init, decay_v, learn_v,
                              learn2_v, c)
            )

    res = run_bass_kernel_spmd(
        nc, in_maps, core_ids=list(range(NCORES)), trace=_trace
    )

    out = np.concatenate(
        [np.asarray(r["out"], dtype=np.float32) for r in res.results], axis=0
    )

    if _trace:
        return out, res
    return out
